# revision 13
# baseline (speedup 1.0000x reference)
"""Trainium2 Bass kernel for BoundaryLoss (data-parallel over batch).

Math (per batch sample b):
  mask  = 5x5-dilate(E) where E marks adjacent differing pixels (E_h|E_v).
          Superset-equal to the reference dilate/erode boundary union: a 5x5
          window is non-uniform iff it contains an adjacent differing pair.
  ce    = logsumexp_c(pred) - pred[t]
  wsum  = sum(mask * ce);  msum = sum(mask)
  per_sample = msum > 0 ? wsum/max(msum,1) : wsum/(H*W);  out = mean_b

Device algorithm (one sample per core):
  - pred streams per class in layout B [128, (4 rows, 512)] (partition p =
    rows 4p..4p+3), 8KB-contiguous DMA runs; the 21 MB fp32 stream is the
    roofline.  Per-class granularity keeps the DMA queues continuously fed.
  - e = exp(pred) on ACT (fp16); S = sum_c e_c via identity-matmul PSUM
    accumulation (4 banks).
  - gather pred[t]: one 4x tensor_scalar is_equal per CLASS PAIR against
    the packed {tb, tb-1} tile (emitted 2 classes ahead so the DVE queue
    never stalls), one 2x mult o = eq*e per class, rows 0-2 accumulate into
    G PSUM (3 banks) via identity matmuls, row 3 into an SBUF fp16
    accumulator (exact: one hit per pixel).
  - mask pipeline (interleaved between classes): E_h = is_ne x-shift in
    layout A; transpose (PE, 1 PSUM bank); E_v = is_ne in transposed space;
    5-row dilate there; transpose back; 5-col dilate; all {0,1} fp16.
  - layout A copies of target/mask move through small row-permuted DRAM
    images so every DMA leg is a hardware-queue shape (no 2KB-run loads).
  - the last class streams as two row-halves so its exp/matmuls overlap the
    final DMA; finals: ln(S), ln(G) fp16 row-split, mask-weighted sums via
    stt accum chasing the lns; msum via ACT accum; partition reduction on
    the host ([128,8] output).
Host combines the per-core outputs.
"""

import numpy as np

B = 8
C = 21
H = 512
W = 512
N_CORES = 8
PW = 516  # padded width for dilate buffers; data cols [2, 514)

_CACHE = {}


def _patch_act_tables(bacc_mod, mybir, arch):
    """Steer the act-table-load pass to the set that contains BOTH exp and
    ln (one table load for the whole kernel instead of a reload between the
    exp stream and the final ln pass): present every other set as empty so
    the greedy selection can only pick the combined one.  Set ids and the
    act_info.json walrus reads stay untouched."""
    try:
        from concourse.hw_specs import get_activation_tables

        orig = get_activation_tables(arch)
        Fn = mybir.ActivationFunctionType
        need = {Fn.Exp, Fn.Ln, Fn.Copy}
        combined = next(name for name, fns in orig.items()
                        if need.issubset(fns))
        tables = {name: (fns if name == combined else set())
                  for name, fns in orig.items()}
        bacc_mod.get_activation_tables = lambda _arch: tables
    except Exception:
        pass


def _build_nc():
    from contextlib import ExitStack

    import concourse.bacc as bacc
    import concourse.tile as tile
    from concourse import mybir
    from concourse.masks import make_identity

    dt = mybir.dt
    Alu = mybir.AluOpType
    Act = mybir.ActivationFunctionType

    nc = bacc.Bacc("TRN2", target_bir_lowering=False, debug=False,
                   num_devices=N_CORES)
    _patch_act_tables(bacc, mybir, nc.m.arch)

    pred = nc.dram_tensor("pred", [C, H, W], dt.float32, kind="ExternalInput")
    target = nc.dram_tensor("target", [H, W], dt.int32, kind="ExternalInput")
    out = nc.dram_tensor("out", [128, 8], dt.float32,
                         kind="ExternalOutput")

    with tile.TileContext(nc) as tc, ExitStack() as ctx:
        consts = ctx.enter_context(tc.tile_pool(name="consts", bufs=1))
        keep = ctx.enter_context(tc.tile_pool(name="keep", bufs=1))
        mp = ctx.enter_context(tc.tile_pool(name="maskpool", bufs=1))
        ms = ctx.enter_context(tc.tile_pool(name="maskscratch", bufs=1))
        ppool = ctx.enter_context(tc.tile_pool(name="pp", bufs=5))
        epool = ctx.enter_context(tc.tile_pool(name="ep", bufs=4))
        qpool = ctx.enter_context(tc.tile_pool(name="qp", bufs=2))
        opool = ctx.enter_context(tc.tile_pool(name="op", bufs=3))
        fin = ctx.enter_context(tc.tile_pool(name="fin", bufs=1))
        dramp = ctx.enter_context(tc.tile_pool(name="dram", bufs=1,
                                               space="DRAM"))
        mps = ctx.enter_context(tc.tile_pool(name="mpsum", bufs=1,
                                             space="PSUM"))
        sgp = ctx.enter_context(tc.tile_pool(name="sgpsum", bufs=1,
                                             space="PSUM"))

        ident = consts.tile([128, 128], dt.float16)
        make_identity(nc, ident)
        warm = consts.tile([128, 512], dt.float16)
        nc.gpsimd.memset(warm, 0.0)
        st = consts.tile([128, 8], dt.float32)
        nc.vector.memset(st, 0.0)

        # ---------------- persistent tensors ----------------
        tbpair = keep.tile([128, 2, 4, W], dt.float16)  # {t, t-1} layout B
        g3d = keep.tile([128, 2, W], dt.float16)        # row-3 gather accum
        maskb = keep.tile([128, 4, W], dt.float16)      # mask (layout B)

        t32b = mp.tile([128, 4, W], dt.int32)
        ta_dram = dramp.tile([H, W], dt.float16)   # row-permuted target img
        mask_dram = dramp.tile([H, W], dt.float16)

        def st_early():
            nc.sync.dma_start(
                out=t32b,
                in_=target.ap().rearrange("(p r) w -> p r w", p=128))
            nc.gpsimd.memset(g3d, 0.0)
            # PE warmup into the future S bank (discarded by c==0 start=True)
            s_ps_w = sgp.tile([128, 4, W], dt.float32, tag="s")
            for _ in range(10):
                nc.tensor.matmul(s_ps_w[:, 0, :], ident, warm, start=True,
                                 stop=True)

        # ---------------- mask pipeline stages ----------------
        tA = mp.tile([128, 4, W], dt.float16)      # target layout A
        EA = mp.tile([128, 4, W], dt.float16)      # E_h (layout A)
        tT = mp.tile([128, 4, W], dt.float16)      # target transposed
        ETp = mp.tile([128, 4, PW], dt.float16)    # E in T space (padded)
        Vd = mp.tile([128, 4, W], dt.float16)      # v-dilated E (T space)
        MAp = mp.tile([128, 4, PW], dt.float16)    # back in A (padded)
        maskA = mp.tile([128, 4, W], dt.float16)

        def st_casts():
            nc.vector.tensor_copy(out=tbpair[:, 0], in_=t32b)
            nc.vector.tensor_scalar(
                out=tbpair[:, 1], in0=tbpair[:, 0], scalar1=1.0, scalar2=None,
                op0=Alu.subtract)
            # B->A via a row-permuted DRAM image: image row 4q+g holds
            # original row 128g+q, so the A-side read is 4KB runs.  The
            # scattered side is the DRAM WRITE (hardware-queue friendly).
            # For p in [32m,32m+32): image_row(4p+r) = 16(p-32m)+4r+m.
            tav = ta_dram[:].rearrange("(a b c) w -> a b c w", a=32, b=4,
                                       c=4)
            for m in range(4):
                nc.gpsimd.dma_start(
                    out=tav[:, :, m, :],
                    in_=tbpair[32 * m:32 * m + 32, 0, :, :])
            nc.gpsimd.dma_start(
                out=tA, in_=ta_dram[:].rearrange("(q r) w -> q r w", q=128))
            nc.gpsimd.memset(ETp, 0.0)
            nc.gpsimd.memset(MAp, 0.0)
            nc.gpsimd.memset(EA[:, :, W - 1:W], 0.0)

        def st_eh():
            nc.vector.tensor_tensor(
                out=EA[:, :, 0:W - 1], in0=tA[:, :, 0:W - 1],
                in1=tA[:, :, 1:W], op=Alu.not_equal)

        def st_tp1():  # transpose tA -> tT
            for q in range(4):
                tq = mps.tile([128, 512], dt.float16, tag="tq")
                for g in range(4):
                    nc.tensor.transpose(
                        tq[:, g * 128:(g + 1) * 128],
                        tA[:, g, q * 128:(q + 1) * 128], ident)
                nc.scalar.copy(out=tT[:, q, :], in_=tq)

        def st_ev():
            nc.vector.tensor_tensor(
                out=ETp[:, :, 2:2 + W - 1], in0=tT[:, :, 0:W - 1],
                in1=tT[:, :, 1:W], op=Alu.not_equal)

        def st_tp2():  # transpose E_h into T space, OR-merge from PSUM
            for q in range(4):
                tq = mps.tile([128, 512], dt.float16, tag="tq")
                for g in range(4):
                    nc.tensor.transpose(
                        tq[:, g * 128:(g + 1) * 128],
                        EA[:, g, q * 128:(q + 1) * 128], ident)
                nc.vector.tensor_tensor(
                    out=ETp[:, q, 2:2 + W], in0=ETp[:, q, 2:2 + W], in1=tq,
                    op=Alu.max)

        def dil_a(src):
            m2 = ms.tile([128, 4, PW], dt.float16, tag="m2")
            nc.vector.tensor_tensor(
                out=m2[:, :, 0:PW - 1],
                in0=src[:, :, 0:PW - 1], in1=src[:, :, 1:PW], op=Alu.max)
            return m2

        def dil_b(src, m2, dst):
            m4 = ms.tile([128, 4, PW], dt.float16, tag="m4")
            nc.vector.tensor_tensor(
                out=m4[:, :, 0:PW - 3],
                in0=m2[:, :, 0:PW - 3], in1=m2[:, :, 2:PW - 1], op=Alu.max)
            nc.vector.tensor_tensor(
                out=dst, in0=m4[:, :, 0:W], in1=src[:, :, 4:4 + W],
                op=Alu.max)

        _scratch = {}

        def st_vdil_a():
            _scratch["m2v"] = dil_a(ETp)

        def st_vdil_b():
            dil_b(ETp, _scratch["m2v"], Vd)

        def st_tp3():  # transpose Vd back to A space
            for g in range(4):
                tg = mps.tile([128, 512], dt.float16, tag="tq")
                for q in range(4):
                    nc.tensor.transpose(
                        tg[:, q * 128:(q + 1) * 128],
                        Vd[:, q, g * 128:(g + 1) * 128], ident)
                nc.scalar.copy(out=MAp[:, g, 2:2 + W], in_=tg)

        def st_hdil_a():
            _scratch["m2h"] = dil_a(MAp)

        def st_hdil_b():
            dil_b(MAp, _scratch["m2h"], maskA)

        def st_bounce():
            nc.gpsimd.dma_start(
                out=mask_dram[:].rearrange("(g p) w -> p g w", p=128),
                in_=maskA)
            nc.gpsimd.dma_start(
                out=maskb,
                in_=mask_dram[:].rearrange("(p r) w -> p r w", p=128))

        def st_msum():
            junk_m = ms.tile([128, 4, W], dt.float16, tag="junkm")
            nc.scalar.activation(out=junk_m, in_=maskA, func=Act.Copy,
                                 accum_out=st[:, 3:4])

        stages = [
            st_casts,
            st_eh,
            st_tp1,
            st_ev,
            st_tp2,
            st_vdil_a,
            st_vdil_b,
            st_tp3,
            st_hdil_a,
            st_hdil_b,
            st_bounce,
            st_msum,
        ]
        stage_i = 0

        def run_stage():
            nonlocal stage_i
            if stage_i < len(stages):
                stages[stage_i]()
                stage_i += 1

        # ---------------- class loop ----------------
        s_ps = sgp.tile([128, 4, W], dt.float32, tag="s")
        g_ps = sgp.tile([128, 3, W], dt.float32, tag="g")

        st_early()

        eq_pairs = {}
        eq20 = mp.tile([128, 4, W], dt.uint16)

        def emit_eq_pair(j):
            eq_t = qpool.tile([128, 2, 4, W], dt.uint16, tag="q")
            nc.vector.tensor_scalar(
                out=eq_t, in0=tbpair, scalar1=float(2 * j),
                scalar2=None, op0=Alu.is_equal)
            eq_pairs[j] = eq_t

        for c in range(C - 1):
            p_t = ppool.tile([128, 4, W], dt.float32, tag="p")
            nc.sync.dma_start(
                out=p_t,
                in_=pred.ap()[c].rearrange("(p r) w -> p r w", p=128))
            run_stage()
            if c == 0:
                emit_eq_pair(0)
            e_t = epool.tile([128, 4, W], dt.float16, tag="e")
            nc.scalar.activation(out=e_t, in_=p_t, func=Act.Exp)
            if c % 2 == 0 and c < 18:
                emit_eq_pair(c // 2 + 1)
            if c == 18:
                nc.vector.tensor_scalar(
                    out=eq20, in0=tbpair[:, 0], scalar1=20.0, scalar2=None,
                    op0=Alu.is_equal)
            o8 = opool.tile([128, 4, W], dt.float16, tag="o")
            eq_t = eq_pairs[c // 2]
            nc.vector.tensor_tensor(out=o8, in0=eq_t[:, c % 2], in1=e_t,
                                    op=Alu.mult)
            nc.vector.tensor_tensor(
                out=g3d[:, c % 2, :], in0=g3d[:, c % 2, :],
                in1=o8[:, 3, :], op=Alu.add)
            for j in range(4):
                nc.tensor.matmul(s_ps[:, j, :], ident, e_t[:, j, :],
                                 start=(c == 0), stop=False)
            for j in range(3):
                nc.tensor.matmul(g_ps[:, j, :], ident, o8[:, j, :],
                                 start=(c == 0), stop=False)
        while stage_i < len(stages):
            run_stage()

        # ---- last class (20) as two row-halves to shorten the tail ----
        p20a = mp.tile([128, 2, W], dt.float32)
        p20b = mp.tile([128, 2, W], dt.float32)
        pv = pred.ap()[C - 1].rearrange("(p r) w -> p r w", p=128)
        nc.sync.dma_start(out=p20a, in_=pv[:, 0:2, :])
        nc.sync.dma_start(out=p20b, in_=pv[:, 2:4, :])
        e20a = mp.tile([128, 2, W], dt.float16)
        e20b = mp.tile([128, 2, W], dt.float16)
        o20a = mp.tile([128, 2, W], dt.float16)
        o20b = mp.tile([128, 2, W], dt.float16)
        nc.scalar.activation(out=e20a, in_=p20a, func=Act.Exp)
        nc.scalar.activation(out=e20b, in_=p20b, func=Act.Exp)
        nc.vector.tensor_tensor(out=o20a, in0=eq20[:, 0:2], in1=e20a,
                                op=Alu.mult)
        for j in range(2):
            nc.tensor.matmul(s_ps[:, j, :], ident, e20a[:, j, :],
                             start=False, stop=True)
            nc.tensor.matmul(g_ps[:, j, :], ident, o20a[:, j, :],
                             start=False, stop=True)
        nc.vector.tensor_tensor(out=o20b, in0=eq20[:, 2:4], in1=e20b,
                                op=Alu.mult)
        nc.vector.tensor_tensor(
            out=g3d[:, 0, :], in0=g3d[:, 0, :], in1=o20b[:, 1, :],
            op=Alu.add)
        for j in range(2, 4):
            nc.tensor.matmul(s_ps[:, j, :], ident, e20b[:, j - 2, :],
                             start=False, stop=True)
        nc.tensor.matmul(g_ps[:, 2, :], ident, o20b[:, 0, :],
                         start=False, stop=True)

        # ---------------- finals ----------------
        l1 = fin.tile([128, 4, W], dt.float16)
        lg = fin.tile([128, 3, W], dt.float16)
        lg3 = fin.tile([128, 1, W], dt.float16)
        g3s = fin.tile([128, W], dt.float16)
        junk = ms.tile([128, 4, W], dt.float16, tag="junkj")

        nc.vector.tensor_tensor(out=g3s, in0=g3d[:, 0, :], in1=g3d[:, 1, :],
                                op=Alu.add)
        nc.scalar.activation(out=l1[:, 0:2], in_=s_ps[:, 0:2], func=Act.Ln)
        nc.scalar.activation(out=l1[:, 2:4], in_=s_ps[:, 2:4], func=Act.Ln)
        nc.vector.scalar_tensor_tensor(
            out=junk[:, 0:2], in0=l1[:, 0:2], scalar=0.0,
            in1=maskb[:, 0:2], op0=Alu.add, op1=Alu.mult,
            accum_out=st[:, 0:1])
        nc.scalar.activation(out=lg, in_=g_ps, func=Act.Ln)
        nc.vector.scalar_tensor_tensor(
            out=junk[:, 2:4], in0=l1[:, 2:4], scalar=0.0,
            in1=maskb[:, 2:4], op0=Alu.add, op1=Alu.mult,
            accum_out=st[:, 1:2])
        nc.scalar.activation(out=lg3, in_=g3s.unsqueeze(1), func=Act.Ln)
        nc.vector.scalar_tensor_tensor(
            out=junk[:, 0:3], in0=lg, scalar=0.0, in1=maskb[:, 0:3],
            op0=Alu.add, op1=Alu.mult, accum_out=st[:, 2:3])
        nc.vector.scalar_tensor_tensor(
            out=junk[:, 3:4], in0=lg3, scalar=0.0, in1=maskb[:, 3:4],
            op0=Alu.add, op1=Alu.mult, accum_out=st[:, 4:5])
        # partition reduction happens on the host: DMA st [128,8] directly
        nc.sync.dma_start(out=out.ap(), in_=st)

    nc.compile()
    return nc


def get_nc():
    if "nc" not in _CACHE:
        _CACHE["nc"] = _build_nc()
    return _CACHE["nc"]


def _combine(outs):
    """outs: list of per-core [128,8] float32 -> scalar loss."""
    per_sample = []
    for o in outs:
        s = o.astype(np.float64).sum(axis=0)
        w1 = s[0] + s[1]
        l2 = s[2] + s[4]
        msum = s[3]
        wsum = w1 - l2
        if msum > 0:
            per_sample.append(wsum / max(msum, 1.0))
        else:
            per_sample.append(wsum / float(H * W))
    return np.float32(np.mean(per_sample))


def kernel(pred, target):
    from concourse.bass_utils import run_bass_kernel_spmd

    pred = np.ascontiguousarray(pred, dtype=np.float32)
    target = np.ascontiguousarray(target, dtype=np.int32)
    assert pred.shape == (B, C, H, W) and target.shape == (B, H, W)

    nc = get_nc()
    in_maps = [{"pred": pred[b], "target": target[b]} for b in range(B)]
    res = run_bass_kernel_spmd(nc, in_maps, core_ids=list(range(N_CORES)))
    outs = [res.results[b]["out"] for b in range(B)]
    return np.asarray(_combine(outs), dtype=np.float32)
